# revision 44
# baseline (speedup 1.0000x reference)
"""BiMambaLayer Trainium2 kernel.

Sharding: 8 cores = batch(2) x direction(2) x head-half(2). Each core runs the
full L=2048 sequence of one (batch, direction) through 16 of the 32 heads of
that direction's Mamba2 block, plus the fused output projection restricted to
its 1024 d_inner channels. The gated-RMSNorm row scale commutes with the output
projections, so each core returns an unnormalized partial projection U and a
partial sum-of-squares s; the host combines:
    out[b] = x[b] + scale * sum_dir flip_d( r_d[:,None] * (U0 + U1) ),
    r_d = rsqrt((s0 + s1)/2048 + eps).

On-chip pipeline per core (channel-major; sequence processed in 4 blocks of
512 positions, scan in 16 chunks of Q=128):
  in_proj (fp PE matmuls, bf16) -> causal depthwise conv as 4 accumulated
  diagonal matmuls on PE + fused silu/softplus on ACT -> chunked SSD scan:
  head-shared C@B^T, per-head segsum decay (DVE diff + ACT exp), masked via
  additive -1e30 before exp, Y = X^T.T@G^T + H^T.T@C''^T accumulated in PSUM,
  chunk-state recurrence on DVE -> gating y*silu(z) (z matmuls deferred to
  late phase to save SBUF) -> U = y_gated @ Mfused^T and sumsq via ones-matmul.
"""
import numpy as np

L = 2048
DM = 1024  # d_model
Q = 128  # scan chunk
NCH = L // Q  # 16 chunks
BLK = 512
NBLK = L // BLK  # 4
CPB = BLK // Q  # 4 chunks per block
NH = 16  # local heads
P = 64  # head dim
NST = 16  # state dim
ECONV = 1056  # xs(1024) + B(16) + C(16)
EPS = 1e-5
NEG = -1e30

_cache = {}


def _build_nc():
    import concourse.bass as bass
    import concourse.tile as tile
    import concourse.mybir as mybir
    from concourse import bacc
    from concourse.masks import make_identity
    from concourse.alu_op_type import AluOpType as alu

    f32 = mybir.dt.float32
    bf16 = mybir.dt.bfloat16
    AF = mybir.ActivationFunctionType

    f8 = mybir.dt.float8e4
    DR = mybir.MatmulPerfMode.DoubleRow

    nc = bacc.Bacc(trn_type="TRN2")

    # ---- DRAM I/O (per-core shapes; host prepares layouts) ----
    xt = nc.dram_tensor("xt", [128, 8, L], bf16, kind="ExternalInput")
    xt8 = nc.dram_tensor("xt8", [128, 8, L], f8, kind="ExternalInput")
    wt8 = nc.dram_tensor("wt8", [128, 8, 2048], f8, kind="ExternalInput")
    wtb = nc.dram_tensor("wtb", [128, 8, 48], bf16, kind="ExternalInput")
    mft = nc.dram_tensor("mft", [128, 8, DM], f8, kind="ExternalInput")
    cwd = nc.dram_tensor("cwd", [128, 36, 128], bf16, kind="ExternalInput")
    oned = nc.dram_tensor("oned", [1, 512], bf16, kind="ExternalInput")
    cb = nc.dram_tensor("cb", [128, 18], f32, kind="ExternalInput")
    hp = nc.dram_tensor("hp", [128, 10], f32, kind="ExternalInput")
    u = nc.dram_tensor("u", [L, DM], bf16, kind="ExternalOutput")
    s = nc.dram_tensor("s", [1, L], f32, kind="ExternalOutput")

    from contextlib import ExitStack

    with tile.TileContext(nc) as tc, ExitStack() as ctx:
        ep = ctx.enter_context
        const = ep(tc.tile_pool(name="const", bufs=1))
        statep = ep(tc.tile_pool(name="state", bufs=1))
        xtp = ep(tc.tile_pool(name="xtp", bufs=2))
        xsrp = ep(tc.tile_pool(name="xsrp", bufs=1))
        xssb = ep(tc.tile_pool(name="xssb", bufs=2))
        zsp = ep(tc.tile_pool(name="zsp", bufs=2))
        dtlp = ep(tc.tile_pool(name="dtlp", bufs=2))
        lrepp = ep(tc.tile_pool(name="lrepp", bufs=1))
        dscrp = ep(tc.tile_pool(name="dscrp", bufs=2, space="DRAM"))
        chkp = ep(tc.tile_pool(name="chkp", bufs=2))
        xstp = ep(tc.tile_pool(name="xstp", bufs=3))
        gpool = ep(tc.tile_pool(name="gp", bufs=2))
        y2p = ep(tc.tile_pool(name="y2p", bufs=2))
        y2blk = ep(tc.tile_pool(name="y2blk", bufs=1))
        pp_mm = ep(tc.tile_pool(name="pp_mm", bufs=2, space="PSUM"))
        pp_ytp = ep(tc.tile_pool(name="pp_ytp", bufs=2, space="PSUM"))
        pp_y = ep(tc.tile_pool(name="pp_y", bufs=2, space="PSUM"))
        pp_gd = ep(tc.tile_pool(name="pp_gd", bufs=1, space="PSUM"))
        pp_s = ep(tc.tile_pool(name="pp_s", bufs=1, space="PSUM"))
        if True:
            # ---------- constants / persistent ----------
            wt8_sb = const.tile([128, 8, 2048], f8)
            nc.sync.dma_start(wt8_sb, wt8[:, :, :])
            wtb_sb = const.tile([128, 8, 48], bf16)
            nc.sync.dma_start(wtb_sb, wtb[:, :, :])
            mft_sb = const.tile([128, 8, DM], f8)
            nc.sync.dma_start(mft_sb, mft[:, :, :])
            cwd_sb = const.tile([128, 36, 128], bf16)
            nc.sync.dma_start(cwd_sb, cwd[:, :, :])
            cb_sb = const.tile([128, 18], f32)
            nc.sync.dma_start(cb_sb, cb[:, :])
            hp_sb = const.tile([128, 10], f32)
            nc.sync.dma_start(hp_sb, hp[:, :])

            ident_b = const.tile([128, 128], bf16)
            make_identity(nc, ident_b)
            ident_f = const.tile([128, 128], f32)
            make_identity(nc, ident_f)
            # multiplicative mask, [j, i] coords: 1 where i >= j, 0 where i < j
            tril01 = const.tile([128, 128], f32)
            nc.gpsimd.memset(tril01, 1.0)
            nc.gpsimd.affine_select(
                out=tril01, in_=tril01, compare_op=mybir.AluOpType.is_ge,
                fill=0.0, base=0, pattern=[[1, 128]], channel_multiplier=-1,
            )
            onesq = const.tile([128, 128], f32)
            nc.vector.memset(onesq, 1.0)
            onescol = const.tile([128, 1], bf16)
            nc.vector.memset(onescol, 1.0)
            # segsum rhs: group g occupies partitions 32g..32g+5: rows
            # 32g+m are head-m indicator rows; 32g+4 holds the chunk's
            # gathered l rows (DMA'd per chunk)
            ind = const.tile([128, 512], bf16)
            nc.vector.memset(ind, 0.0)
            for g in range(4):
                isl = ind[32 * g : 32 * g + 4, :]
                nc.gpsimd.memset(isl, 1.0)
                nc.gpsimd.affine_select(
                    out=isl, in_=isl, compare_op=mybir.AluOpType.is_ge,
                    fill=0.0, base=0, pattern=[[1, 512]], channel_multiplier=-128,
                )
                nc.gpsimd.affine_select(
                    out=isl, in_=isl, compare_op=mybir.AluOpType.is_ge,
                    fill=0.0, base=127, pattern=[[-1, 512]], channel_multiplier=128,
                )

            # chunk-decay per-partition scalars, [state-tile, chunk]
            texp_st = statep.tile([128, 4, NCH], f32, tag="texp")
            nc.vector.memset(texp_st, 0.0)
            # paired rhs for the fused Y DoubleRow matmul: per head h,
            # slot 0 = gsb (per chunk), slot 1 = zero-padded C'' (per block;
            # only rows 32k..32k+16 ever written)
            gct = statep.tile([128, NH, 2, BLK], f8, tag="gct")
            nc.vector.memset(gct, 0.0)
            # paired lhsT: slots 0..15 = xpos per head, 16..23 = state slabs
            # (each t duplicated so head pairs can share a 128-wide k-tile)
            xst_cur = xstp.tile([128, 24, P], f8, tag="xst")
            nc.vector.memset(xst_cur[:, 16:24, :], 0.0)

            halo3 = statep.tile([128, 8, 3], bf16, tag="halo3")
            bch3 = statep.tile([32, 3], bf16, tag="bch3")

            for b in range(NBLK):
                bsl = slice(b * BLK, (b + 1) * BLK)
                # ---------- load x block ----------
                xtb = xtp.tile([128, 8, BLK], bf16, tag="xtb")
                nc.sync.dma_start(xtb, xt[:, :, bsl])
                xtb8 = xtp.tile([128, 8, BLK], f8, tag="xtb8")
                nc.sync.dma_start(xtb8, xt8[:, :, bsl])

                # ---------- in_proj: xs tiles (fp8 DoubleRow, 8x-scaled) ----
                xsr = xsrp.tile([128, 8, BLK + 3], bf16, tag="xsr")
                bcr = xsrp.tile([32, BLK + 3], bf16, tag="bcr")
                dt_sp = dtlp.tile([128, BLK], f32, tag="dtsp")
                for et in range(8):
                    ecol = et * 128
                    ps = pp_mm.tile([128, BLK], f32, tag="mm")
                    for kp in range(4):
                        nc.tensor.matmul(
                            ps, wt8_sb[:, 2 * kp : 2 * kp + 2, ecol : ecol + 128],
                            xtb8[:, 2 * kp : 2 * kp + 2, :],
                            start=(kp == 0), stop=(kp == 3), perf_mode=DR,
                        )
                    nc.scalar.activation(
                        xsr[:, et, 3 : 3 + BLK], ps, AF.Copy, scale=0.125
                    )
                # BCdt (bf16, exact)
                ps = pp_mm.tile([128, BLK], f32, tag="mm")
                for kt in range(8):
                    nc.tensor.matmul(
                        ps[:48, :], wtb_sb[:, kt, :], xtb[:, kt, :],
                        start=(kt == 0), stop=(kt == 7),
                    )
                nc.scalar.copy(bcr[:, 3 : 3 + BLK], ps[0:32, :])
                nc.scalar.activation(
                    dt_sp[32:48, :], ps[32:48, :], AF.Exp,
                    bias=hp_sb[32:48, 0:1],
                )
                nc.vector.tensor_scalar_add(
                    dt_sp[32:48, :], dt_sp[32:48, :], 1.0
                )
                nc.scalar.activation(dt_sp[32:48, :], dt_sp[32:48, :], AF.Ln)
                # conv left halo from the previous block's tail
                if b == 0:
                    nc.vector.memset(xsr[:, :, 0:3], 0.0)
                    nc.vector.memset(bcr[:, 0:3], 0.0)
                else:
                    nc.vector.tensor_copy(xsr[:, :, 0:3], halo3)
                    nc.vector.tensor_copy(bcr[:, 0:3], bch3)
                if b < NBLK - 1:
                    nc.vector.tensor_copy(halo3, xsr[:, :, BLK : BLK + 3])
                    nc.vector.tensor_copy(bch3, bcr[:, BLK : BLK + 3])

                # ---------- conv (bf16 diag matmuls) + silu ----------
                xs_sb = xssb.tile([128, 8, BLK], bf16, tag="xs")
                bct = dtlp.tile([32, BLK], bf16, tag="bct")
                ct4 = dtlp.tile([128, BLK], bf16, tag="ct4")
                for ct in range(9):
                    m = 128 if ct < 8 else 32
                    src = xsr[:, ct, :] if ct < 8 else bcr
                    ps = pp_mm.tile([128, BLK], f32, tag="mm")
                    for k in range(4):
                        nc.tensor.matmul(
                            ps[:m, :], cwd_sb[:m, ct * 4 + k, :m], src[:m, k : k + BLK],
                            start=(k == 0), stop=(k == 3),
                        )
                    # silu(u) = u * (0.5*tanh(u/2) + 0.5), u = conv + cb
                    dst = xs_sb[:, ct, :] if ct < 8 else bct
                    xu = y2p.tile([128, BLK], bf16, tag="xu")
                    nc.scalar.activation(
                        xu[:m, :], ps[:m, :], AF.Identity,
                        bias=cb_sb[:m, ct : ct + 1],
                    )
                    th = y2p.tile([128, BLK], bf16, tag="th")
                    nc.scalar.activation(
                        th[:m, :], ps[:m, :], AF.Tanh,
                        bias=cb_sb[:m, 9 + ct : 10 + ct], scale=0.5,
                    )
                    nc.vector.scalar_tensor_tensor(
                        dst, th[:m, :], 1.0, xu[:m, :], alu.add, alu.mult
                    )
                # C rows replicated to the four 32-aligned bases
                for k4 in range(4):
                    nc.sync.dma_start(ct4[32 * k4 : 32 * k4 + 16, :], bct[16:32, :])

                # ---------- z in_proj (fp8 DR, 8x) + silu ----------
                zs = zsp.tile([128, 8, BLK], bf16, tag="zs")
                for zt in range(8):
                    ps = pp_mm.tile([128, BLK], f32, tag="mm")
                    for kp in range(4):
                        nc.tensor.matmul(
                            ps, wt8_sb[:, 2 * kp : 2 * kp + 2, 1024 + zt * 128 : 1152 + zt * 128],
                            xtb8[:, 2 * kp : 2 * kp + 2, :],
                            start=(kp == 0), stop=(kp == 3), perf_mode=DR,
                        )
                    # zs = 0.25*silu(z): the 0.25 keeps gated-y inside fp8 range
                    # (compensated by 32x mft weights + 1/32 in the U copy and
                    # scale=4 in the sumsq square)
                    zt_t = y2p.tile([128, BLK], bf16, tag="th")
                    nc.scalar.activation(zt_t, ps, AF.Tanh, scale=0.0625)
                    nc.gpsimd.tensor_scalar(zt_t, zt_t, 0.015625, 0.015625, alu.mult, alu.add)
                    nc.vector.tensor_tensor(zs[:, zt, :], zt_t, ps, alu.mult)

                # ---------- dt pipeline ----------
                dtA = lrepp.tile([128, BLK], f32, tag="dtA")
                lcm = dtlp.tile([128, BLK], f32, tag="lcm")  # cumsum l, channel-major
                wc2 = dtlp.tile([128, BLK], bf16, tag="wc2")  # exp(l)
                nc.vector.tensor_scalar_mul(
                    dtA[32:48, :], dt_sp[32:48, :], hp_sb[32:48, 1:2]
                )
                for cc in range(CPB):
                    qs = slice(cc * Q, (cc + 1) * Q)
                    nc.vector.tensor_tensor_scan(
                        lcm[32:48, qs], onesq[32:48, :], dtA[32:48, qs],
                        0.0, alu.mult, alu.add,
                    )
                # clamp: bounds the masked-region segsum so exp stays finite
                # (valid-region decays below e^-60 are ~0 anyway)
                nc.vector.tensor_scalar(
                    lcm[32:48, :], lcm[32:48, :], -60.0, None, alu.max
                )
                nc.scalar.activation(wc2[32:48, :], lcm[32:48, :], AF.Exp)
                # chunk total decays: exp(l[:, last-of-chunk])
                texp_cm = dtlp.tile([128, CPB, 1], f32, tag="texpcm")
                lv = lcm[32:48, :].rearrange("p (c q) -> p c q", q=Q)
                nc.scalar.activation(texp_cm[32:48, :, :], lv[:, :, 127:128], AF.Exp)
                # bf16 +/- l rows (kept at partitions 32:48 for alignment)
                lbf = dtlp.tile([48, BLK], bf16, tag="lbf")
                nc.scalar.copy(lbf[32:48, :], lcm[32:48, :])
                lnbf = dtlp.tile([48, BLK], bf16, tag="lnbf")
                nc.scalar.activation(lnbf[32:48, :], lcm[32:48, :], AF.Copy, scale=-1.0)
                # bounce small per-block vectors through DRAM so they can be
                # partition-broadcast on the way back in
                dscr = dscrp.tile([48, BLK], f32, tag="dscr")
                dscr2 = dscrp.tile([16, BLK], bf16, tag="dscr2")
                dscrl = dscrp.tile([16, BLK], bf16, tag="dscrl")
                dscrn = dscrp.tile([16, BLK], bf16, tag="dscrn")
                nc.sync.dma_start(dscr[0:16, :], lcm[32:48, :])
                nc.sync.dma_start(dscr2, wc2[32:48, :])
                nc.sync.dma_start(dscrl, lbf[32:48, :])
                nc.sync.dma_start(dscrn, lnbf[32:48, :])
                # segsum lhsT rows per group g (partitions 32g..32g+5):
                # [-l_{4g}..-l_{4g+3}; ones]
                nl = dtlp.tile([128, BLK], bf16, tag="nl")
                for g in range(4):
                    nc.sync.dma_start(
                        nl[32 * g : 32 * g + 4, :],
                        bass.AP(dscrn.tensor, dscrn.offset + 4 * g * BLK,
                                [[BLK, 4], [1, BLK]]),
                    )
                _oap = oned[:, :]
                nc.sync.dma_start(
                    bass.AP(nl.tensor, nl.offset + 4 * BLK,
                            [[32 * BLK, 4], [1, BLK]]),
                    bass.AP(_oap.tensor, _oap.offset, [[0, 4], [1, BLK]]),
                )
                nc.sync.dma_start(
                    dscr[32:48, 0:CPB],
                    texp_cm[32:48, :, :].rearrange("p c one -> p (c one)"),
                )
                for t in range(4):
                    for k in range(4):
                        nc.sync.dma_start(
                            texp_st[32 * k : 32 * k + 16, t, b * CPB : (b + 1) * CPB],
                            bass.AP(dscr.tensor,
                                    dscr.offset + (32 + 4 * t + k) * BLK,
                                    [[0, 16], [1, CPB]]),
                        )

                # ---------- C'' = C * exp(l_h) per head ----------
                wc2rep = lrepp.tile([128, 4, BLK], bf16, tag="wc2rep")
                for k in range(4):
                    nc.sync.dma_start(
                        wc2rep[32 * k : 32 * k + 16, :, :],
                        bass.AP(dscr2.tensor, dscr2.offset + k * BLK,
                                [[0, 16], [4 * BLK, 4], [1, BLK]]),
                    )
                for h in range(NH):
                    k, t = h % 4, h // 4
                    nc.gpsimd.tensor_tensor(
                        gct[32 * k : 32 * k + 16, h, 1, :],
                        ct4[32 * k : 32 * k + 16, :],
                        wc2rep[32 * k : 32 * k + 16, t, :],
                        alu.mult,
                    )

                # ---------- per-chunk scan ----------
                y2 = y2blk.tile([128, 8, BLK], bf16, tag="y2")
                ysb8 = y2blk.tile([128, 8, BLK], f8, tag="ysb8")
                ublk = y2blk.tile([128, CPB, DM], bf16, tag="ublk")
                for cc in range(CPB):
                    c = b * CPB + cc
                    qs = slice(cc * Q, (cc + 1) * Q)
                    # small transposes + S psum share one bank
                    sps = pp_s.tile([128, 512], f32, tag="sps")
                    nc.vector.memset(sps[:, 0:256], 0.0)
                    nc.tensor.transpose(sps[:, 256:272], dt_sp[32:48, qs], ident_f[32:48, 32:48])
                    nc.tensor.transpose(sps[:, 272:288], lcm[32:48, qs], ident_f[32:48, 32:48])
                    dtc = chkp.tile([128, NST], f32, tag="dtc")
                    lc = chkp.tile([128, NST], f32, tag="lc")
                    nc.vector.tensor_copy(dtc, sps[:, 256:272])
                    nc.vector.tensor_copy(lc, sps[:, 272:288])
                    # B position-major (bf16 view of spare sps columns)
                    bpp = sps.bitcast(bf16)
                    nc.tensor.transpose(bpp[:, 576:592], bct[0:16, qs], ident_b[0:16, 0:16])
                    bpos = chkp.tile([128, NST], bf16, tag="bpos")
                    nc.vector.tensor_copy(bpos, bpp[:, 576:592])
                    # X^T via PE transpose, dt_j scaling fused into the
                    # psum->sbuf move on DVE (into the paired xst tile)
                    for w in range(2):
                        tp = pp_ytp.tile([128, 512], f32, tag="ytp")
                        tpb = tp.bitcast(bf16)
                        for ct in range(4):
                            nc.tensor.transpose(
                                tpb[:, ct * 128 : ct * 128 + 128],
                                xs_sb[:, w * 4 + ct, qs], ident_b,
                            )
                        nc.vector.tensor_tensor(
                            xst_cur[:, 8 * w : 8 * w + 8, :],
                            tpb[:, 0:512].rearrange("p (h c) -> p h c", c=P),
                            dtc[:, 8 * w : 8 * w + 8]
                            .rearrange("p (h one) -> p h one", one=1)
                            .to_broadcast([128, 8, P]),
                            alu.mult,
                        )
                    # wS = dt * exp(T - l) (position-major), then B_ws
                    wscol = chkp.tile([128, NH], f32, tag="wscol")
                    trep = chkp.tile([128, NH], f32, tag="trep")
                    nc.sync.dma_start(
                        trep,
                        bass.AP(dscr.tensor, dscr.offset + cc * Q + 127,
                                [[0, 128], [BLK, NH]]),
                    )
                    nc.vector.tensor_tensor(wscol, trep, lc, alu.subtract)
                    nc.scalar.activation(wscol, wscol, AF.Exp)
                    bws = chkp.tile([128, NH, NST], f8, tag="bws")
                    nc.vector.tensor_tensor(
                        bws,
                        bpos.rearrange("p (one n) -> p one n", one=1).to_broadcast([128, NH, NST]),
                        wscol.rearrange("p (h one) -> p h one", one=1).to_broadcast([128, NH, NST]),
                        alu.mult,
                    )
                    # head-shared C@B^T -> G0^T[j, i], pre-masked (spare sps cols)
                    nc.tensor.matmul(
                        sps[:, 384:512], bct[0:16, qs], ct4[0:16, qs], start=True, stop=True
                    )
                    g0m = chkp.tile([128, Q], f32, tag="g0m")
                    nc.vector.tensor_tensor(g0m, sps[:, 384:512], tril01, alu.mult)
                    # segsum decay via PE rank-2 matmuls:
                    # gd[j, (m, i)] = l_h[i] - l_h[j], h = 4g + m
                    # chunk's l rows -> partitions 32g+4 of ind (one strided DMA)
                    nc.sync.dma_start(
                        bass.AP(ind.tensor, ind.offset + 4 * 512,
                                [[32 * 512, 4], [128, 4], [1, Q]]),
                        bass.AP(dscrl.tensor, dscrl.offset + cc * Q,
                                [[4 * BLK, 4], [BLK, 4], [1, Q]]),
                    )
                    gdec = gpool.tile([128, NH, Q], bf16, tag="gdec")
                    for g in range(4):
                        hsl = slice(g * 4, g * 4 + 4)
                        gd = pp_gd.tile([128, 512], f32, tag="gd")
                        nc.tensor.matmul(
                            gd, nl[32 * g : 32 * g + 5, qs],
                            ind[32 * g : 32 * g + 5, :],
                            start=True, stop=True, tile_position=(32 * g, 0),
                        )
                        nc.scalar.activation(
                            gdec[:, hsl, :],
                            gd.rearrange("p (h i) -> p h i", i=Q), AF.Exp,
                        )
                        # masked G rows into the paired-rhs slot 0
                        nc.vector.tensor_tensor(
                            gct[:, hsl, 0, qs],
                            g0m.rearrange("p (one i) -> p one i", one=1)
                            .to_broadcast([128, 4, Q]),
                            gdec[:, hsl, :], alu.mult,
                        )
                    # per-quad Y banks; every head's fused DR matmul writes a
                    # full 128-row block (its own 64 channel rows are valid,
                    # the other 64 are junk) so gating partitions line up
                    _xp = 24 * P
                    _gp = NH * 2 * BLK
                    for qd in range(4):
                        yq = pp_y.tile([128, 512], f32, tag="yq")
                        for hi in range(4):
                            h = 4 * qd + hi
                            k, t = h % 4, h // 4
                            # S^T = B_ws.T @ X_h
                            nc.tensor.matmul(
                                sps[32 * k : 32 * k + 16, t * 64 : t * 64 + 64],
                                bws[:, h, :], xst_cur[:, h, :],
                                start=True, stop=True, tile_position=(0, 32 * k),
                            )
                            # fused fp8 DR: Y^T = X_h.T @ G^T + st_t.T @ C''pad^T
                            lhs = bass.AP(
                                xst_cur.tensor,
                                xst_cur.offset + (h - h % 2) * P,
                                [[_xp, 128],
                                 [(16 + 2 * t - (h - h % 2)) * P, 2], [1, 2 * P]],
                            )
                            rhs = bass.AP(
                                gct.tensor, gct.offset + h * 2 * BLK + cc * Q,
                                [[_gp, 128], [BLK, 2], [1, Q]],
                            )
                            nc.tensor.matmul(
                                yq[:, hi * 128 : hi * 128 + 128], lhs, rhs,
                                start=True, stop=True, perf_mode=DR,
                            )
                        # gating for the quad's two pairs, split by head half
                        for half in range(2):
                            hsl2 = slice(64 * half, 64 * half + 64)
                            for pj in range(2):
                                pr = 2 * qd + pj
                                col = (2 * pj + half) * 128
                                tmp = gpool.tile([128, 128], f32, tag="gtmp")
                                nc.vector.scalar_tensor_tensor(
                                    tmp[hsl2, :], xs_sb[hsl2, pr, qs],
                                    hp_sb[hsl2, 2 + pr : 3 + pr],
                                    yq[hsl2, col : col + 128],
                                    alu.mult, alu.add,
                                )
                                nc.vector.tensor_tensor(
                                    ysb8[hsl2, pr, qs], tmp[hsl2, :],
                                    zs[hsl2, pr, qs], alu.mult,
                                )
                    # state recurrence into the next chunk's paired tile
                    xst_next = xstp.tile([128, 24, P], f8, tag="xst")
                    for t in range(4):
                        nc.vector.scalar_tensor_tensor(
                            xst_next[:, 16 + 2 * t : 18 + 2 * t, :],
                            xst_cur[:, 16 + 2 * t : 18 + 2 * t, :],
                            texp_st[:, t, c : c + 1],
                            sps[:, t * 64 : t * 64 + 64]
                            .rearrange("p (one x) -> p one x", one=1)
                            .to_broadcast([128, 2, P]),
                            alu.mult, alu.add,
                        )
                    xst_cur = xst_next
                    # ---------- U matmuls (fp8 DR, 32x weights) ----------
                    for oc in range(2):
                        ups = pp_mm.tile([128, BLK], f32, tag="mm")
                        for cp in range(4):
                            nc.tensor.matmul(
                                ups, ysb8[:, 2 * cp : 2 * cp + 2, qs],
                                mft_sb[:, 2 * cp : 2 * cp + 2, oc * 512 : oc * 512 + 512],
                                start=(cp == 0), stop=(cp == 3), perf_mode=DR,
                            )
                        nc.scalar.activation(
                            ublk[:, cc, oc * 512 : oc * 512 + 512], ups, AF.Copy,
                            scale=0.03125,
                        )
                # batched u store for the block
                nc.sync.dma_start(
                    u[bsl, :].rearrange("(cc p) d -> p cc d", p=128), ublk
                )
                # block sumsq (batched squares, then ones-matmul)
                for pr in range(8):
                    nc.scalar.activation(
                        y2[:, pr, :], ysb8[:, pr, :], AF.Square, scale=4.0
                    )
                ssps = pp_mm.tile([128, BLK], f32, tag="mm")
                for ct in range(8):
                    nc.tensor.matmul(
                        ssps[0:1, :], onescol, y2[:, ct, :],
                        start=(ct == 0), stop=(ct == 7),
                    )
                ssb = y2p.tile([1, BLK], f32, tag="ssb")
                nc.vector.tensor_copy(ssb, ssps[0:1, :])
                nc.sync.dma_start(s[0:1, bsl], ssb)

    nc.finalize()
    return nc


def _get_nc():
    if "nc" not in _cache:
        _cache["nc"] = _build_nc()
    return _cache["nc"]


def _prep_core_inputs(inputs, b, d, hh):
    import ml_dtypes

    bf16 = ml_dtypes.bfloat16
    f8 = ml_dtypes.float8_e4m3fn
    pre = "fwd" if d == 0 else "bwd"
    W = np.asarray(inputs[f"{pre}_in_proj_w"], np.float32)  # (4160, 1024)
    x = np.asarray(inputs["x"], np.float32)[b]  # (L, 1024)
    if d == 1:
        x = x[::-1]
    # x^T as (128, 8, L)
    xtv = np.ascontiguousarray(x.T.reshape(8, 128, L).transpose(1, 0, 2))
    # fp8 weights: [xs 1024 | z 1024] columns, 8x-scaled
    W_xs = W[2048 + hh * 1024 : 3072 + hh * 1024]
    W_B = W[4096:4112]
    W_C = W[4112:4128]
    W_dt = W[4128 + hh * 16 : 4144 + hh * 16]
    W_z = W[hh * 1024 : 1024 + hh * 1024]
    Wt8 = np.concatenate([W_xs, W_z], axis=0).T * 8.0  # (1024, 2048)
    wt8v = np.ascontiguousarray(Wt8.reshape(8, 128, 2048).transpose(1, 0, 2))
    Wtb = np.concatenate([W_B, W_C, W_dt], axis=0).T  # (1024, 48)
    wtbv = np.ascontiguousarray(Wtb.reshape(8, 128, 48).transpose(1, 0, 2))
    # fused output projection
    Wo = np.asarray(inputs[f"{pre}_out_proj_w"], np.float32)  # (1024, 2048)
    Wl = np.asarray(inputs["layer_out_proj_w"], np.float32)  # (1024, 2048)
    nw = np.asarray(inputs[f"{pre}_norm_w"], np.float32)
    ch = slice(hh * 1024, hh * 1024 + 1024)
    M = (Wl[:, d * 1024 : d * 1024 + 1024] @ Wo)[:, ch] * nw[ch][None, :]
    M = M * 0.125  # xs path carries 2x from the fused silu; y_g is 8x
    MfT = M.T * 32.0  # (1024 c, 1024 o), 32x for fp8 (undone in U copy)
    mftv = np.ascontiguousarray(MfT.reshape(8, 128, 1024).transpose(1, 0, 2))
    # conv: dense diagonal weight tiles (fp8 for xs, bf16 for BC)
    cwf = np.asarray(inputs[f"{pre}_conv_w"], np.float32)[:, 0, :]  # (2080, 4)
    cwl = np.concatenate([cwf[hh * 1024 : 1024 + hh * 1024], cwf[2048:2080]], axis=0)
    cwdv = np.zeros((128, 36, 128), np.float32)
    for ct in range(9):
        n = 128 if ct < 8 else 32
        ii = np.arange(n)
        for k in range(4):
            cwdv[ii, ct * 4 + k, ii] = cwl[ct * 128 : ct * 128 + n, k]
    cbf = np.asarray(inputs[f"{pre}_conv_b"], np.float32)
    cbl = np.concatenate([cbf[hh * 1024 : 1024 + hh * 1024], cbf[2048:2080]])
    cbv = np.zeros((128, 18), np.float32)
    for ct in range(9):
        n = 128 if ct < 8 else 32
        cbv[:n, ct] = cbl[ct * 128 : ct * 128 + n]
        cbv[:n, 9 + ct] = 0.5 * cbl[ct * 128 : ct * 128 + n]
    # host params
    hpv = np.zeros((128, 10), np.float32)
    hs = slice(hh * 16, hh * 16 + 16)
    hpv[32:48, 0] = np.asarray(inputs[f"{pre}_dt_bias"], np.float32)[hs]
    hpv[32:48, 1] = -np.exp(np.asarray(inputs[f"{pre}_A_log"], np.float32)[hs])
    Dp = np.asarray(inputs[f"{pre}_Dp"], np.float32)[hs]
    for pr in range(8):
        rows = (np.arange(128) + pr * 128) // 64  # local head of channel
        hpv[:, 2 + pr] = 4.0 * Dp[rows]  # match the 8x gated-y scaling
    return {
        "xt": xtv.astype(bf16),
        "xt8": xtv.astype(f8),
        "wt8": wt8v.astype(f8),
        "wtb": wtbv.astype(bf16),
        "mft": mftv.astype(f8),
        "cwd": cwdv.astype(bf16),
        "oned": np.ones((1, 512), np.float32).astype(bf16),
        "cb": cbv,
        "hp": hpv,
    }


def _combine(inputs, results):
    x = np.asarray(inputs["x"], np.float32)
    scale = np.asarray(inputs["layer_scale"], np.float32)
    out = x.copy()
    i = 0
    for b in range(2):
        for d in range(2):
            U0 = np.asarray(results[i]["u"], np.float32)
            s0 = results[i]["s"][0]
            U1 = np.asarray(results[i + 1]["u"], np.float32)
            s1 = results[i + 1]["s"][0]
            i += 2
            r = 1.0 / np.sqrt((s0 + s1) / 64.0 / 2048.0 + EPS)
            contrib = r[:, None] * (U0 + U1)
            if d == 1:
                contrib = contrib[::-1]
            out[b] += contrib * scale[None, :]
    return out


def _run(inputs, trace=False, core_ids=None):
    from concourse.bass_utils import run_bass_kernel_spmd

    nc = _get_nc()
    in_maps = []
    for b in range(2):
        for d in range(2):
            for hh in range(2):
                in_maps.append(_prep_core_inputs(inputs, b, d, hh))
    if core_ids is None:
        core_ids = list(range(8))
    res = run_bass_kernel_spmd(
        nc, in_maps[: len(core_ids)], core_ids=core_ids, trace=trace
    )
    return res


def kernel(**inputs):
    res = _run(inputs)
    return _combine(inputs, res.results)



# revision 47
# speedup vs baseline: 1.0975x; 1.0975x over previous
"""BiMambaLayer Trainium2 kernel.

Sharding: 8 cores = batch(2) x direction(2) x head-half(2). Each core runs the
full L=2048 sequence of one (batch, direction) through 16 of the 32 heads of
that direction's Mamba2 block, plus the fused output projection restricted to
its 1024 d_inner channels. The gated-RMSNorm row scale commutes with the output
projections, so each core returns an unnormalized partial projection U and a
partial sum-of-squares s; the host combines:
    out[b] = x[b] + scale * sum_dir flip_d( r_d[:,None] * (U0 + U1) ),
    r_d = rsqrt((s0 + s1)/2048 + eps).

On-chip pipeline per core (channel-major; sequence processed in 4 blocks of
512 positions, scan in 16 chunks of Q=128):
  in_proj (fp PE matmuls, bf16) -> causal depthwise conv as 4 accumulated
  diagonal matmuls on PE + fused silu/softplus on ACT -> chunked SSD scan:
  head-shared C@B^T, per-head segsum decay (DVE diff + ACT exp), masked via
  additive -1e30 before exp, Y = X^T.T@G^T + H^T.T@C''^T accumulated in PSUM,
  chunk-state recurrence on DVE -> gating y*silu(z) (z matmuls deferred to
  late phase to save SBUF) -> U = y_gated @ Mfused^T and sumsq via ones-matmul.
"""
import numpy as np

L = 2048
DM = 1024  # d_model
Q = 128  # scan chunk
NCH = L // Q  # 16 chunks
BLK = 512
NBLK = L // BLK  # 4
CPB = BLK // Q  # 4 chunks per block
NH = 16  # local heads
P = 64  # head dim
NST = 16  # state dim
ECONV = 1056  # xs(1024) + B(16) + C(16)
EPS = 1e-5
NEG = -1e30

_cache = {}


def _build_nc():
    import concourse.bass as bass
    import concourse.tile as tile
    import concourse.mybir as mybir
    from concourse import bacc
    from concourse.masks import make_identity
    from concourse.alu_op_type import AluOpType as alu

    f32 = mybir.dt.float32
    bf16 = mybir.dt.bfloat16
    AF = mybir.ActivationFunctionType

    f8 = mybir.dt.float8e4
    DR = mybir.MatmulPerfMode.DoubleRow

    nc = bacc.Bacc(trn_type="TRN2")

    # ---- DRAM I/O (per-core shapes; host prepares layouts) ----
    xt = nc.dram_tensor("xt", [128, 8, L], bf16, kind="ExternalInput")
    xt8 = nc.dram_tensor("xt8", [128, 8, L], f8, kind="ExternalInput")
    wt8 = nc.dram_tensor("wt8", [128, 8, 2048], f8, kind="ExternalInput")
    wtb = nc.dram_tensor("wtb", [128, 8, 48], bf16, kind="ExternalInput")
    mft = nc.dram_tensor("mft", [128, 8, DM], f8, kind="ExternalInput")
    cwd = nc.dram_tensor("cwd", [128, 36, 128], bf16, kind="ExternalInput")
    oned = nc.dram_tensor("oned", [1, 512], bf16, kind="ExternalInput")
    cb = nc.dram_tensor("cb", [128, 18], f32, kind="ExternalInput")
    hp = nc.dram_tensor("hp", [128, 10], f32, kind="ExternalInput")
    u = nc.dram_tensor("u", [L, DM], bf16, kind="ExternalOutput")
    s = nc.dram_tensor("s", [1, L], f32, kind="ExternalOutput")

    from contextlib import ExitStack

    with tile.TileContext(nc) as tc, ExitStack() as ctx:
        ep = ctx.enter_context
        const = ep(tc.tile_pool(name="const", bufs=1))
        statep = ep(tc.tile_pool(name="state", bufs=1))
        xtp = ep(tc.tile_pool(name="xtp", bufs=2))
        xsrp = ep(tc.tile_pool(name="xsrp", bufs=1))
        xssb = ep(tc.tile_pool(name="xssb", bufs=2))
        zsp = ep(tc.tile_pool(name="zsp", bufs=2))
        dtlp = ep(tc.tile_pool(name="dtlp", bufs=2))
        lrepp = ep(tc.tile_pool(name="lrepp", bufs=1))
        dscrp = ep(tc.tile_pool(name="dscrp", bufs=2, space="DRAM"))
        chkp = ep(tc.tile_pool(name="chkp", bufs=2))
        xstp = ep(tc.tile_pool(name="xstp", bufs=3))
        gpool = ep(tc.tile_pool(name="gp", bufs=2))
        y2p = ep(tc.tile_pool(name="y2p", bufs=2))
        y2blk = ep(tc.tile_pool(name="y2blk", bufs=1))
        pp_mm = ep(tc.tile_pool(name="pp_mm", bufs=2, space="PSUM"))
        pp_ytp = ep(tc.tile_pool(name="pp_ytp", bufs=2, space="PSUM"))
        pp_y = ep(tc.tile_pool(name="pp_y", bufs=2, space="PSUM"))
        pp_gd = ep(tc.tile_pool(name="pp_gd", bufs=1, space="PSUM"))
        pp_s = ep(tc.tile_pool(name="pp_s", bufs=1, space="PSUM"))
        if True:
            # ---------- constants / persistent ----------
            wt8_sb = const.tile([128, 8, 2048], f8)
            nc.sync.dma_start(wt8_sb, wt8[:, :, :])
            wtb_sb = const.tile([128, 8, 48], bf16)
            nc.sync.dma_start(wtb_sb, wtb[:, :, :])
            mft_sb = const.tile([128, 8, DM], f8)
            nc.sync.dma_start(mft_sb, mft[:, :, :])
            cwd_sb = const.tile([128, 36, 128], bf16)
            nc.sync.dma_start(cwd_sb, cwd[:, :, :])
            cb_sb = const.tile([128, 18], f32)
            nc.sync.dma_start(cb_sb, cb[:, :])
            hp_sb = const.tile([128, 10], f32)
            nc.sync.dma_start(hp_sb, hp[:, :])

            ident_b = const.tile([128, 128], bf16)
            make_identity(nc, ident_b)
            ident_f = const.tile([128, 128], f32)
            make_identity(nc, ident_f)
            # multiplicative mask, [j, i] coords: 1 where i >= j, 0 where i < j
            tril01 = const.tile([128, 128], f32)
            nc.gpsimd.memset(tril01, 1.0)
            nc.gpsimd.affine_select(
                out=tril01, in_=tril01, compare_op=mybir.AluOpType.is_ge,
                fill=0.0, base=0, pattern=[[1, 128]], channel_multiplier=-1,
            )
            onesq = const.tile([128, 128], f32)
            nc.vector.memset(onesq, 1.0)
            onescol = const.tile([128, 1], bf16)
            nc.vector.memset(onescol, 1.0)
            # segsum rhs: group g occupies partitions 32g..32g+5: rows
            # 32g+m are head-m indicator rows; 32g+4 holds the chunk's
            # gathered l rows (DMA'd per chunk)
            ind = const.tile([128, 512], bf16)
            nc.vector.memset(ind, 0.0)
            for g in range(4):
                isl = ind[32 * g : 32 * g + 4, :]
                nc.gpsimd.memset(isl, 1.0)
                nc.gpsimd.affine_select(
                    out=isl, in_=isl, compare_op=mybir.AluOpType.is_ge,
                    fill=0.0, base=0, pattern=[[1, 512]], channel_multiplier=-128,
                )
                nc.gpsimd.affine_select(
                    out=isl, in_=isl, compare_op=mybir.AluOpType.is_ge,
                    fill=0.0, base=127, pattern=[[-1, 512]], channel_multiplier=128,
                )

            # chunk-decay per-partition scalars, [state-tile, chunk]
            texp_st = statep.tile([128, 4, NCH], f32, tag="texp")
            nc.vector.memset(texp_st, 0.0)
            # paired rhs for the fused Y DoubleRow matmul: per head h,
            # slot 0 = gsb (per chunk), slot 1 = zero-padded C'' (per block;
            # only rows 32k..32k+16 ever written)
            gct = statep.tile([128, NH, 2, BLK], f8, tag="gct")
            nc.gpsimd.memset(gct, 0.0)
            # paired lhsT: slots 0..15 = xpos per head, 16..23 = state slabs
            # (each t duplicated so head pairs can share a 128-wide k-tile)
            xst_cur = xstp.tile([128, 24, P], f8, tag="xst")
            nc.vector.memset(xst_cur[:, 16:24, :], 0.0)

            halo3 = statep.tile([128, 8, 3], bf16, tag="halo3")
            bch3 = statep.tile([32, 3], bf16, tag="bch3")

            for b in range(NBLK):
                bsl = slice(b * BLK, (b + 1) * BLK)
                # ---------- load x block ----------
                xtb = xtp.tile([128, 8, BLK], bf16, tag="xtb")
                nc.sync.dma_start(xtb, xt[:, :, bsl])
                xtb8 = xtp.tile([128, 8, BLK], f8, tag="xtb8")
                nc.sync.dma_start(xtb8, xt8[:, :, bsl])

                # ---------- in_proj: xs tiles (fp8 DoubleRow, 8x-scaled) ----
                xsr = xsrp.tile([128, 8, BLK + 3], bf16, tag="xsr")
                bcr = xsrp.tile([32, BLK + 3], bf16, tag="bcr")
                dt_sp = dtlp.tile([128, BLK], f32, tag="dtsp")
                for et in range(8):
                    ecol = et * 128
                    ps = pp_mm.tile([128, BLK], f32, tag="mm")
                    for kp in range(4):
                        nc.tensor.matmul(
                            ps, wt8_sb[:, 2 * kp : 2 * kp + 2, ecol : ecol + 128],
                            xtb8[:, 2 * kp : 2 * kp + 2, :],
                            start=(kp == 0), stop=(kp == 3), perf_mode=DR,
                        )
                    nc.scalar.activation(
                        xsr[:, et, 3 : 3 + BLK], ps, AF.Copy, scale=0.125
                    )
                # BCdt (bf16, exact)
                ps = pp_mm.tile([128, BLK], f32, tag="mm")
                for kt in range(8):
                    nc.tensor.matmul(
                        ps[:48, :], wtb_sb[:, kt, :], xtb[:, kt, :],
                        start=(kt == 0), stop=(kt == 7),
                    )
                nc.scalar.copy(bcr[:, 3 : 3 + BLK], ps[0:32, :])
                nc.scalar.activation(
                    dt_sp[32:48, :], ps[32:48, :], AF.Exp,
                    bias=hp_sb[32:48, 0:1],
                )
                nc.vector.tensor_scalar_add(
                    dt_sp[32:48, :], dt_sp[32:48, :], 1.0
                )
                nc.scalar.activation(dt_sp[32:48, :], dt_sp[32:48, :], AF.Ln)
                # conv left halo from the previous block's tail
                if b == 0:
                    nc.vector.memset(xsr[:, :, 0:3], 0.0)
                    nc.vector.memset(bcr[:, 0:3], 0.0)
                else:
                    nc.vector.tensor_copy(xsr[:, :, 0:3], halo3)
                    nc.vector.tensor_copy(bcr[:, 0:3], bch3)
                if b < NBLK - 1:
                    nc.vector.tensor_copy(halo3, xsr[:, :, BLK : BLK + 3])
                    nc.vector.tensor_copy(bch3, bcr[:, BLK : BLK + 3])

                # ---------- conv (bf16 diag matmuls) + silu ----------
                xs_sb = xssb.tile([128, 8, BLK], bf16, tag="xs")
                bct = dtlp.tile([32, BLK], bf16, tag="bct")
                ct4 = dtlp.tile([128, BLK], bf16, tag="ct4")
                for ct in range(9):
                    m = 128 if ct < 8 else 32
                    src = xsr[:, ct, :] if ct < 8 else bcr
                    ps = pp_mm.tile([128, BLK], f32, tag="mm")
                    for k in range(4):
                        nc.tensor.matmul(
                            ps[:m, :], cwd_sb[:m, ct * 4 + k, :m], src[:m, k : k + BLK],
                            start=(k == 0), stop=(k == 3),
                        )
                    # silu(u) = u * (0.5*tanh(u/2) + 0.5), u = conv + cb
                    dst = xs_sb[:, ct, :] if ct < 8 else bct
                    xu = y2p.tile([128, BLK], bf16, tag="xu")
                    nc.scalar.activation(
                        xu[:m, :], ps[:m, :], AF.Identity,
                        bias=cb_sb[:m, ct : ct + 1],
                    )
                    th = y2p.tile([128, BLK], bf16, tag="th")
                    nc.scalar.activation(
                        th[:m, :], ps[:m, :], AF.Tanh,
                        bias=cb_sb[:m, 9 + ct : 10 + ct], scale=0.5,
                    )
                    nc.vector.scalar_tensor_tensor(
                        dst, th[:m, :], 1.0, xu[:m, :], alu.add, alu.mult
                    )
                # C rows replicated to the four 32-aligned bases
                for k4 in range(4):
                    nc.sync.dma_start(ct4[32 * k4 : 32 * k4 + 16, :], bct[16:32, :])

                # ---------- z in_proj (fp8 DR, 8x) + silu ----------
                zs = zsp.tile([128, 8, BLK], bf16, tag="zs")
                for zt in range(8):
                    ps = pp_mm.tile([128, BLK], f32, tag="mm")
                    for kp in range(4):
                        nc.tensor.matmul(
                            ps, wt8_sb[:, 2 * kp : 2 * kp + 2, 1024 + zt * 128 : 1152 + zt * 128],
                            xtb8[:, 2 * kp : 2 * kp + 2, :],
                            start=(kp == 0), stop=(kp == 3), perf_mode=DR,
                        )
                    # zs = 0.25*silu(z): the 0.25 keeps gated-y inside fp8 range
                    # (compensated by 32x mft weights + 1/32 in the U copy and
                    # scale=4 in the sumsq square)
                    zt_t = y2p.tile([128, BLK], bf16, tag="th")
                    nc.scalar.activation(zt_t, ps, AF.Tanh, scale=0.0625)
                    nc.gpsimd.tensor_scalar(zt_t, zt_t, 0.015625, 0.015625, alu.mult, alu.add)
                    nc.vector.tensor_tensor(zs[:, zt, :], zt_t, ps, alu.mult)

                # ---------- dt pipeline ----------
                dtA = lrepp.tile([128, BLK], f32, tag="dtA")
                lcm = dtlp.tile([128, BLK], f32, tag="lcm")  # cumsum l, channel-major
                wc2 = dtlp.tile([128, BLK], bf16, tag="wc2")  # exp(l)
                nc.vector.tensor_scalar_mul(
                    dtA[32:48, :], dt_sp[32:48, :], hp_sb[32:48, 1:2]
                )
                for cc in range(CPB):
                    qs = slice(cc * Q, (cc + 1) * Q)
                    nc.vector.tensor_tensor_scan(
                        lcm[32:48, qs], onesq[32:48, :], dtA[32:48, qs],
                        0.0, alu.mult, alu.add,
                    )
                # clamp: bounds the masked-region segsum so exp stays finite
                # (valid-region decays below e^-60 are ~0 anyway)
                nc.vector.tensor_scalar(
                    lcm[32:48, :], lcm[32:48, :], -60.0, None, alu.max
                )
                nc.scalar.activation(wc2[32:48, :], lcm[32:48, :], AF.Exp)
                # chunk total decays: exp(l[:, last-of-chunk])
                texp_cm = dtlp.tile([128, CPB, 1], f32, tag="texpcm")
                lv = lcm[32:48, :].rearrange("p (c q) -> p c q", q=Q)
                nc.scalar.activation(texp_cm[32:48, :, :], lv[:, :, 127:128], AF.Exp)
                # bf16 +/- l rows (kept at partitions 32:48 for alignment)
                lbf = dtlp.tile([48, BLK], bf16, tag="lbf")
                nc.scalar.copy(lbf[32:48, :], lcm[32:48, :])
                lnbf = dtlp.tile([48, BLK], bf16, tag="lnbf")
                nc.scalar.activation(lnbf[32:48, :], lcm[32:48, :], AF.Copy, scale=-1.0)
                # bounce small per-block vectors through DRAM so they can be
                # partition-broadcast on the way back in
                dscr = dscrp.tile([48, BLK], f32, tag="dscr")
                dscr2 = dscrp.tile([16, BLK], bf16, tag="dscr2")
                dscrl = dscrp.tile([16, BLK], bf16, tag="dscrl")
                dscrn = dscrp.tile([16, BLK], bf16, tag="dscrn")
                nc.sync.dma_start(dscr[0:16, :], lcm[32:48, :])
                nc.sync.dma_start(dscr2, wc2[32:48, :])
                nc.sync.dma_start(dscrl, lbf[32:48, :])
                nc.sync.dma_start(dscrn, lnbf[32:48, :])
                # segsum lhsT rows per group g (partitions 32g..32g+5):
                # [-l_{4g}..-l_{4g+3}; ones]
                nl = dtlp.tile([128, BLK], bf16, tag="nl")
                for g in range(4):
                    nc.sync.dma_start(
                        nl[32 * g : 32 * g + 4, :],
                        bass.AP(dscrn.tensor, dscrn.offset + 4 * g * BLK,
                                [[BLK, 4], [1, BLK]]),
                    )
                _oap = oned[:, :]
                nc.sync.dma_start(
                    bass.AP(nl.tensor, nl.offset + 4 * BLK,
                            [[32 * BLK, 4], [1, BLK]]),
                    bass.AP(_oap.tensor, _oap.offset, [[0, 4], [1, BLK]]),
                )
                nc.sync.dma_start(
                    dscr[32:48, 0:CPB],
                    texp_cm[32:48, :, :].rearrange("p c one -> p (c one)"),
                )
                for t in range(4):
                    for k in range(4):
                        nc.sync.dma_start(
                            texp_st[32 * k : 32 * k + 16, t, b * CPB : (b + 1) * CPB],
                            bass.AP(dscr.tensor,
                                    dscr.offset + (32 + 4 * t + k) * BLK,
                                    [[0, 16], [1, CPB]]),
                        )

                # ---------- C'' = C * exp(l_h) per head ----------
                wc2rep = lrepp.tile([128, 4, BLK], bf16, tag="wc2rep")
                for k in range(4):
                    nc.sync.dma_start(
                        wc2rep[32 * k : 32 * k + 16, :, :],
                        bass.AP(dscr2.tensor, dscr2.offset + k * BLK,
                                [[0, 16], [4 * BLK, 4], [1, BLK]]),
                    )
                for h in range(NH):
                    k, t = h % 4, h // 4
                    nc.gpsimd.tensor_tensor(
                        gct[32 * k : 32 * k + 16, h, 1, :],
                        ct4[32 * k : 32 * k + 16, :],
                        wc2rep[32 * k : 32 * k + 16, t, :],
                        alu.mult,
                    )

                # ---------- per-chunk scan ----------
                y2 = y2blk.tile([128, 8, BLK], bf16, tag="y2")
                ysb8 = y2blk.tile([128, 8, BLK], f8, tag="ysb8")
                ublk = y2blk.tile([128, CPB, DM], bf16, tag="ublk")
                # Dp*xs, block-wide (lets the per-chunk gating be scalar-free)
                dpx = y2blk.tile([128, 8, BLK], bf16, tag="dpx")
                for pr in range(8):
                    nc.vector.tensor_scalar_mul(
                        dpx[:, pr, :], xs_sb[:, pr, :], hp_sb[:, 2 + pr : 3 + pr]
                    )
                for cc in range(CPB):
                    c = b * CPB + cc
                    qs = slice(cc * Q, (cc + 1) * Q)
                    # small transposes + S psum share one bank
                    sps = pp_s.tile([128, 512], f32, tag="sps")
                    nc.vector.memset(sps[:, 0:256], 0.0)
                    nc.tensor.transpose(sps[:, 256:272], dt_sp[32:48, qs], ident_f[32:48, 32:48])
                    nc.tensor.transpose(sps[:, 272:288], lcm[32:48, qs], ident_f[32:48, 32:48])
                    dtc = chkp.tile([128, NST], f32, tag="dtc")
                    lc = chkp.tile([128, NST], f32, tag="lc")
                    nc.vector.tensor_copy(dtc, sps[:, 256:272])
                    nc.vector.tensor_copy(lc, sps[:, 272:288])
                    # B position-major (bf16 view of spare sps columns)
                    bpp = sps.bitcast(bf16)
                    nc.tensor.transpose(bpp[:, 576:592], bct[0:16, qs], ident_b[0:16, 0:16])
                    bpos = chkp.tile([128, NST], bf16, tag="bpos")
                    nc.vector.tensor_copy(bpos, bpp[:, 576:592])
                    # X^T via PE transpose, dt_j scaling fused into the
                    # psum->sbuf move on DVE (into the paired xst tile)
                    for w in range(2):
                        tp = pp_ytp.tile([128, 512], f32, tag="ytp")
                        tpb = tp.bitcast(bf16)
                        for ct in range(4):
                            nc.tensor.transpose(
                                tpb[:, ct * 128 : ct * 128 + 128],
                                xs_sb[:, w * 4 + ct, qs], ident_b,
                            )
                        nc.vector.tensor_tensor(
                            xst_cur[:, 8 * w : 8 * w + 8, :],
                            tpb[:, 0:512].rearrange("p (h c) -> p h c", c=P),
                            dtc[:, 8 * w : 8 * w + 8]
                            .rearrange("p (h one) -> p h one", one=1)
                            .to_broadcast([128, 8, P]),
                            alu.mult,
                        )
                    # wS = dt * exp(T - l) (position-major), then B_ws
                    wscol = chkp.tile([128, NH], f32, tag="wscol")
                    trep = chkp.tile([128, NH], f32, tag="trep")
                    nc.sync.dma_start(
                        trep,
                        bass.AP(dscr.tensor, dscr.offset + cc * Q + 127,
                                [[0, 128], [BLK, NH]]),
                    )
                    nc.vector.tensor_tensor(wscol, trep, lc, alu.subtract)
                    nc.scalar.activation(wscol, wscol, AF.Exp)
                    bws = chkp.tile([128, NH, NST], f8, tag="bws")
                    nc.vector.tensor_tensor(
                        bws,
                        bpos.rearrange("p (one n) -> p one n", one=1).to_broadcast([128, NH, NST]),
                        wscol.rearrange("p (h one) -> p h one", one=1).to_broadcast([128, NH, NST]),
                        alu.mult,
                    )
                    # head-shared C@B^T -> G0^T[j, i], pre-masked (spare sps cols)
                    nc.tensor.matmul(
                        sps[:, 384:512], bct[0:16, qs], ct4[0:16, qs], start=True, stop=True
                    )
                    g0m = chkp.tile([128, Q], f32, tag="g0m")
                    nc.vector.tensor_tensor(g0m, sps[:, 384:512], tril01, alu.mult)
                    # segsum decay via PE rank-2 matmuls:
                    # gd[j, (m, i)] = l_h[i] - l_h[j], h = 4g + m
                    # chunk's l rows -> partitions 32g+4 of ind (one strided DMA)
                    nc.sync.dma_start(
                        bass.AP(ind.tensor, ind.offset + 4 * 512,
                                [[32 * 512, 4], [128, 4], [1, Q]]),
                        bass.AP(dscrl.tensor, dscrl.offset + cc * Q,
                                [[4 * BLK, 4], [BLK, 4], [1, Q]]),
                    )
                    gdec = gpool.tile([128, NH, Q], bf16, tag="gdec")
                    for g in range(4):
                        hsl = slice(g * 4, g * 4 + 4)
                        gd = pp_gd.tile([128, 512], f32, tag="gd")
                        nc.tensor.matmul(
                            gd, nl[32 * g : 32 * g + 5, qs],
                            ind[32 * g : 32 * g + 5, :],
                            start=True, stop=True, tile_position=(32 * g, 0),
                        )
                        nc.scalar.activation(
                            gdec[:, hsl, :],
                            gd.rearrange("p (h i) -> p h i", i=Q), AF.Exp,
                        )
                        # masked G rows into the paired-rhs slot 0
                        nc.vector.tensor_tensor(
                            gct[:, hsl, 0, qs],
                            g0m.rearrange("p (one i) -> p one i", one=1)
                            .to_broadcast([128, 4, Q]),
                            gdec[:, hsl, :], alu.mult,
                        )
                    # per-quad Y banks; every head's fused DR matmul writes a
                    # full 128-row block (its own 64 channel rows are valid,
                    # the other 64 are junk) so gating partitions line up
                    _xp = 24 * P
                    _gp = NH * 2 * BLK
                    for qd in range(4):
                        yq = pp_y.tile([128, 512], f32, tag="yq")
                        for hi in range(4):
                            h = 4 * qd + hi
                            k, t = h % 4, h // 4
                            # S^T = B_ws.T @ X_h
                            nc.tensor.matmul(
                                sps[32 * k : 32 * k + 16, t * 64 : t * 64 + 64],
                                bws[:, h, :], xst_cur[:, h, :],
                                start=True, stop=True, tile_position=(0, 32 * k),
                            )
                            # fused fp8 DR: Y^T = X_h.T @ G^T + st_t.T @ C''pad^T
                            lhs = bass.AP(
                                xst_cur.tensor,
                                xst_cur.offset + (h - h % 2) * P,
                                [[_xp, 128],
                                 [(16 + 2 * t - (h - h % 2)) * P, 2], [1, 2 * P]],
                            )
                            rhs = bass.AP(
                                gct.tensor, gct.offset + h * 2 * BLK + cc * Q,
                                [[_gp, 128], [BLK, 2], [1, Q]],
                            )
                            nc.tensor.matmul(
                                yq[:, hi * 128 : hi * 128 + 128], lhs, rhs,
                                start=True, stop=True, perf_mode=DR,
                            )
                        # gating for the quad's two pairs, batched per half:
                        # (dpx + Y) * zs over the two pair-columns at once
                        for half in range(2):
                            hsl2 = slice(64 * half, 64 * half + 64)
                            tmp = gpool.tile([128, 2, 128], f32, tag="gtmp")
                            yqv = bass.AP(
                                yq.tensor, yq.offset + 64 * half * 512 + half * 128,
                                [[512, 64], [256, 2], [1, 128]],
                            )
                            nc.vector.tensor_tensor(
                                tmp[hsl2, :, :],
                                dpx[hsl2, 2 * qd : 2 * qd + 2, qs],
                                yqv, alu.add,
                            )
                            nc.vector.tensor_tensor(
                                ysb8[hsl2, 2 * qd : 2 * qd + 2, qs],
                                tmp[hsl2, :, :],
                                zs[hsl2, 2 * qd : 2 * qd + 2, qs], alu.mult,
                            )
                    # state recurrence into the next chunk's paired tile
                    xst_next = xstp.tile([128, 24, P], f8, tag="xst")
                    for t in range(4):
                        nc.vector.scalar_tensor_tensor(
                            xst_next[:, 16 + 2 * t : 18 + 2 * t, :],
                            xst_cur[:, 16 + 2 * t : 18 + 2 * t, :],
                            texp_st[:, t, c : c + 1],
                            sps[:, t * 64 : t * 64 + 64]
                            .rearrange("p (one x) -> p one x", one=1)
                            .to_broadcast([128, 2, P]),
                            alu.mult, alu.add,
                        )
                    xst_cur = xst_next
                    # ---------- U matmuls (fp8 DR, 32x weights) ----------
                    for oc in range(2):
                        ups = pp_mm.tile([128, BLK], f32, tag="mm")
                        for cp in range(4):
                            nc.tensor.matmul(
                                ups, ysb8[:, 2 * cp : 2 * cp + 2, qs],
                                mft_sb[:, 2 * cp : 2 * cp + 2, oc * 512 : oc * 512 + 512],
                                start=(cp == 0), stop=(cp == 3), perf_mode=DR,
                            )
                        nc.scalar.activation(
                            ublk[:, cc, oc * 512 : oc * 512 + 512], ups, AF.Copy,
                            scale=0.03125,
                        )
                # batched u store for the block
                nc.sync.dma_start(
                    u[bsl, :].rearrange("(cc p) d -> p cc d", p=128), ublk
                )
                # block sumsq (batched squares, then ones-matmul)
                for pr in range(8):
                    nc.scalar.activation(
                        y2[:, pr, :], ysb8[:, pr, :], AF.Square, scale=4.0
                    )
                ssps = pp_mm.tile([128, BLK], f32, tag="mm")
                for ct in range(8):
                    nc.tensor.matmul(
                        ssps[0:1, :], onescol, y2[:, ct, :],
                        start=(ct == 0), stop=(ct == 7),
                    )
                ssb = y2p.tile([1, BLK], f32, tag="ssb")
                nc.vector.tensor_copy(ssb, ssps[0:1, :])
                nc.sync.dma_start(s[0:1, bsl], ssb)

    nc.finalize()
    return nc


def _get_nc():
    if "nc" not in _cache:
        _cache["nc"] = _build_nc()
    return _cache["nc"]


def _prep_core_inputs(inputs, b, d, hh):
    import ml_dtypes

    bf16 = ml_dtypes.bfloat16
    f8 = ml_dtypes.float8_e4m3fn
    pre = "fwd" if d == 0 else "bwd"
    W = np.asarray(inputs[f"{pre}_in_proj_w"], np.float32)  # (4160, 1024)
    x = np.asarray(inputs["x"], np.float32)[b]  # (L, 1024)
    if d == 1:
        x = x[::-1]
    # x^T as (128, 8, L)
    xtv = np.ascontiguousarray(x.T.reshape(8, 128, L).transpose(1, 0, 2))
    # fp8 weights: [xs 1024 | z 1024] columns, 8x-scaled
    W_xs = W[2048 + hh * 1024 : 3072 + hh * 1024]
    W_B = W[4096:4112]
    W_C = W[4112:4128]
    W_dt = W[4128 + hh * 16 : 4144 + hh * 16]
    W_z = W[hh * 1024 : 1024 + hh * 1024]
    Wt8 = np.concatenate([W_xs, W_z], axis=0).T * 8.0  # (1024, 2048)
    wt8v = np.ascontiguousarray(Wt8.reshape(8, 128, 2048).transpose(1, 0, 2))
    Wtb = np.concatenate([W_B, W_C, W_dt], axis=0).T  # (1024, 48)
    wtbv = np.ascontiguousarray(Wtb.reshape(8, 128, 48).transpose(1, 0, 2))
    # fused output projection
    Wo = np.asarray(inputs[f"{pre}_out_proj_w"], np.float32)  # (1024, 2048)
    Wl = np.asarray(inputs["layer_out_proj_w"], np.float32)  # (1024, 2048)
    nw = np.asarray(inputs[f"{pre}_norm_w"], np.float32)
    ch = slice(hh * 1024, hh * 1024 + 1024)
    M = (Wl[:, d * 1024 : d * 1024 + 1024] @ Wo)[:, ch] * nw[ch][None, :]
    M = M * 0.125  # xs path carries 2x from the fused silu; y_g is 8x
    MfT = M.T * 32.0  # (1024 c, 1024 o), 32x for fp8 (undone in U copy)
    mftv = np.ascontiguousarray(MfT.reshape(8, 128, 1024).transpose(1, 0, 2))
    # conv: dense diagonal weight tiles (fp8 for xs, bf16 for BC)
    cwf = np.asarray(inputs[f"{pre}_conv_w"], np.float32)[:, 0, :]  # (2080, 4)
    cwl = np.concatenate([cwf[hh * 1024 : 1024 + hh * 1024], cwf[2048:2080]], axis=0)
    cwdv = np.zeros((128, 36, 128), np.float32)
    for ct in range(9):
        n = 128 if ct < 8 else 32
        ii = np.arange(n)
        for k in range(4):
            cwdv[ii, ct * 4 + k, ii] = cwl[ct * 128 : ct * 128 + n, k]
    cbf = np.asarray(inputs[f"{pre}_conv_b"], np.float32)
    cbl = np.concatenate([cbf[hh * 1024 : 1024 + hh * 1024], cbf[2048:2080]])
    cbv = np.zeros((128, 18), np.float32)
    for ct in range(9):
        n = 128 if ct < 8 else 32
        cbv[:n, ct] = cbl[ct * 128 : ct * 128 + n]
        cbv[:n, 9 + ct] = 0.5 * cbl[ct * 128 : ct * 128 + n]
    # host params
    hpv = np.zeros((128, 10), np.float32)
    hs = slice(hh * 16, hh * 16 + 16)
    hpv[32:48, 0] = np.asarray(inputs[f"{pre}_dt_bias"], np.float32)[hs]
    hpv[32:48, 1] = -np.exp(np.asarray(inputs[f"{pre}_A_log"], np.float32)[hs])
    Dp = np.asarray(inputs[f"{pre}_Dp"], np.float32)[hs]
    for pr in range(8):
        rows = (np.arange(128) + pr * 128) // 64  # local head of channel
        hpv[:, 2 + pr] = 4.0 * Dp[rows]  # match the 8x gated-y scaling
    return {
        "xt": xtv.astype(bf16),
        "xt8": xtv.astype(f8),
        "wt8": wt8v.astype(f8),
        "wtb": wtbv.astype(bf16),
        "mft": mftv.astype(f8),
        "cwd": cwdv.astype(bf16),
        "oned": np.ones((1, 512), np.float32).astype(bf16),
        "cb": cbv,
        "hp": hpv,
    }


def _combine(inputs, results):
    x = np.asarray(inputs["x"], np.float32)
    scale = np.asarray(inputs["layer_scale"], np.float32)
    out = x.copy()
    i = 0
    for b in range(2):
        for d in range(2):
            U0 = np.asarray(results[i]["u"], np.float32)
            s0 = results[i]["s"][0]
            U1 = np.asarray(results[i + 1]["u"], np.float32)
            s1 = results[i + 1]["s"][0]
            i += 2
            r = 1.0 / np.sqrt((s0 + s1) / 64.0 / 2048.0 + EPS)
            contrib = r[:, None] * (U0 + U1)
            if d == 1:
                contrib = contrib[::-1]
            out[b] += contrib * scale[None, :]
    return out


def _run(inputs, trace=False, core_ids=None):
    from concourse.bass_utils import run_bass_kernel_spmd

    nc = _get_nc()
    in_maps = []
    for b in range(2):
        for d in range(2):
            for hh in range(2):
                in_maps.append(_prep_core_inputs(inputs, b, d, hh))
    if core_ids is None:
        core_ids = list(range(8))
    res = run_bass_kernel_spmd(
        nc, in_maps[: len(core_ids)], core_ids=core_ids, trace=trace
    )
    return res


def kernel(**inputs):
    res = _run(inputs)
    return _combine(inputs, res.results)

